# revision 27
# baseline (speedup 1.0000x reference)
"""Trainium2 Bass kernel for nn_CETLayer (GNN message passing + per-node softmax).

Strategy (8 NeuronCores, no collectives needed):
  - Shard edges BY DST RANGE: core c owns nodes [c*2560, (c+1)*2560) and all
    edges pointing into that range. Each core computes its output slice fully
    locally; host concatenates.
  - Host prep: sort edges by dst, group into 128-node "groups" (20 per core),
    pad each group's edge list to KT*128 (KT = global max) so all cores run
    the identical (SPMD) program. The per-edge dst one-hot matrices S
    ([128 edges, 128 nodes] bf16 per tile) are precomputed host-side (a pure
    relayout of the dst index tensor) and DMA'd in, rather than generated
    on-device (the DVE is_equal path measured ~290ns/tile plus long
    pool-reuse stalls and was the top engine).
  - Device per edge-tile [128 edges]:
      gather src rows (gpsimd dma_gather, bf16),
      msg = gath + edge on DVE (bf16 2x mode, whole group in one op),
      one PE transpose-matmul (lhsT=msg, rhs=ident) -> msgT PSUM,
      one PE agg matmul (lhsT=msg again - shared stationary, rhs=S) -> aggT,
      relu (ACT, per 4-tile quad) -> hT, p1 = hT.T @ W.T (PE, emitted one
      quad late so the ACT relu hides under the next quad's matmuls),
      e1 = exp(p1) (ACT, group-level), pe1 = p1*e1 (DVE),
      stats matmul (lhsT=e1pe1 [e,17] - 17-col stationary, rhs=S) -> statsT
      accumulating sum(e1), sum(p1*e1), deg over the group's tiles.
  - Softmax max-subtraction is skipped (mathematically identical, values are
    O(1) so exp() is safe in f32), and the fc bias b is algebraically pulled
    out of the softmax (weights are shift-invariant; sum(w)=1 puts +b back
    at the end): s = (sum(p1'*e1') + p2'*e2')/(sum(e1')+e2') + b.
  - Group finalize computes p2 path, softmax combine, sigmoid (via tanh,
    which shares the ACT table with exp/relu/copy), deg>0 mask. Divisions
    use reciprocal_approx_fast (~18 bits, 5x faster than DVE reciprocal).

kernel(**inputs) takes the FULL inputs and returns the FULL [20000, 8] f32
output. Compute runs in bf16 on the PE (f32 PSUM accumulation).
"""

import dataclasses

import numpy as np
import ml_dtypes

import concourse.bacc as bacc
import concourse.mybir as mybir
import concourse.tile as tile

P = 128
N_NODES = 20000
D = 128
T = 8
CORES = 8
G = 20                      # groups (of 128 nodes) per core
NODES_PER_CORE = G * P      # 2560
N_PAD = CORES * NODES_PER_CORE  # 20480
N_GROUPS = CORES * G        # 160
PAD_DST = 384.0             # out of [0,128) -> zero one-hot row (exact in bf16)

BF16 = ml_dtypes.bfloat16

_BUILD_CACHE: dict[int, object] = {}
LAST_RESULT = None  # BassKernelResults of the most recent run (for test.py)


def _build(KT: int):
    """Build + compile the SPMD program for KT edge-tiles per node-group."""
    L = KT * P  # padded edges per group
    f32 = mybir.dt.float32
    bf16 = mybir.dt.bfloat16
    i16 = mybir.dt.int16
    AOP = mybir.AluOpType
    AF = mybir.ActivationFunctionType

    nc = bacc.Bacc("TRN2", target_bir_lowering=False, enable_partition_id=False,
                   num_swdge_queues=4)

    fp8 = mybir.dt.float8e4
    edge_d = nc.dram_tensor("edge", [P, G, KT, D], bf16, kind="ExternalInput")
    onehot_d = nc.dram_tensor("onehot", [P, G, KT, P], fp8, kind="ExternalInput")
    srcidx_d = nc.dram_tensor("srcidx", [P, G * L // 16], i16, kind="ExternalInput")
    srcemb_d = nc.dram_tensor("srcemb", [N_NODES, D], bf16, kind="ExternalInput")
    ident_d = nc.dram_tensor("ident", [P, P], bf16, kind="ExternalInput")
    wt_d = nc.dram_tensor("wt", [D, T], bf16, kind="ExternalInput")
    ones18_d = nc.dram_tensor("ones18", [1, T], bf16, kind="ExternalInput")
    bhalf_d = nc.dram_tensor("bhalf", [T, 1], f32, kind="ExternalInput")
    out_d = nc.dram_tensor("out", [T, G * P], f32, kind="ExternalOutput")

    with tile.TileContext(nc) as tc, nc.allow_low_precision(reason="bf16 compute"):
        with (
            tc.tile_pool(name="const", bufs=1) as cpool,
            tc.tile_pool(name="edges", bufs=2) as epool,
            tc.tile_pool(name="onehot", bufs=2) as opool,
            tc.tile_pool(name="gath", bufs=2) as gpool,
            tc.tile_pool(name="msg", bufs=2) as mpool,
            tc.tile_pool(name="work", bufs=2) as wpool,
            tc.tile_pool(name="fin", bufs=2) as fpool,
            tc.tile_pool(name="ps_msg", bufs=2, space="PSUM") as ps_msg,
            tc.tile_pool(name="ps_p1", bufs=2, space="PSUM") as ps_p1,
            tc.tile_pool(name="ps_acc", bufs=2, space="PSUM") as ps_acc,
            tc.tile_pool(name="ps_fin", bufs=1, space="PSUM") as ps_fin,
        ):
            # ---- gather indices first: the Q7 gather stream paces the whole
            # kernel, so its inputs must land before anything else ----
            srcidx_all = cpool.tile([P, G * L // 16], i16)
            nc.sync.dma_start(out=srcidx_all[:, 0:L // 16],
                              in_=srcidx_d[:, 0:L // 16])
            nc.sync.dma_start(out=srcidx_all[:, L // 16:],
                              in_=srcidx_d[:, L // 16:])
            # ---- constants / whole-run SBUF residents ----
            ident = cpool.tile([P, P], bf16)
            nc.sync.dma_start(out=ident[:], in_=ident_d[:])
            wt = cpool.tile([D, T], bf16)
            nc.sync.dma_start(out=wt[:], in_=wt_d[:])
            ones18 = cpool.tile([1, T], bf16)
            nc.sync.dma_start(out=ones18[:], in_=ones18_d[:])
            bhalf = cpool.tile([T, 1], f32)
            nc.sync.dma_start(out=bhalf[:], in_=bhalf_d[:])
            out_all = cpool.tile([T, G * P], f32)
            # batched-finalize accumulators (whole run)
            NW = G * P  # 2560 nodes per core
            stats_all = cpool.tile([17, NW], f32)
            p2_all = cpool.tile([T, NW], f32)

            CH = 512
            CG = CH // P  # groups per finalize chunk

            def finalize_chunk(f0, CH):
                fs = slice(f0, f0 + CH)
                # partition-shifting moves must go through (SBUF->SBUF) DMA
                spe1 = fpool.tile([T, CH], f32, tag="spe1")
                nc.sync.dma_start(out=spe1[:], in_=stats_all[T:2 * T, fs])
                degrow = fpool.tile([1, CH], f32, tag="degrow")
                nc.sync.dma_start(out=degrow[:], in_=stats_all[16:17, fs])
                # r = 1/max(deg,1) ; halfmask = 0.5*(deg > 0)
                dmax = fpool.tile([1, CH], f32, tag="dmax")
                nc.vector.tensor_scalar(out=dmax[:], in0=degrow[:],
                                        scalar1=1.0, scalar2=None, op0=AOP.max)
                rm = fpool.tile([1, 2 * CH], f32, tag="rm")
                nc.vector.reciprocal_approx_fast(out=rm[:, 0:CH], in_=dmax[:])
                nc.vector.tensor_scalar(out=rm[:, CH:2 * CH], in0=degrow[:],
                                        scalar1=0.0, scalar2=0.5,
                                        op0=AOP.is_gt, op1=AOP.mult)
                # broadcast r|halfmask to the 8 type-partitions via PE
                rm_bf = fpool.tile([1, 2 * CH], bf16, tag="rm_bf")
                nc.vector.tensor_scalar(out=rm_bf[:], in0=rm[:], scalar1=0.0,
                                        scalar2=None, op0=AOP.add)
                bc_sb = fpool.tile([T, 2 * CH], f32, tag="bc_sb")
                for c0 in (0, CH):
                    bc = ps_fin.tile([T, CH], f32, tag="p2T")
                    nc.tensor.matmul(out=bc[:], lhsT=ones18[:],
                                     rhs=rm_bf[:, c0:c0 + CH], start=True, stop=True)
                    nc.scalar.activation(out=bc_sb[:, c0:c0 + CH], in_=bc[:],
                                         func=AF.Copy)
                r_bc = bc_sb[:, 0:CH]
                hm_bc = bc_sb[:, CH:2 * CH]
                p2s = fpool.tile([T, CH], f32, tag="p2s")
                nc.vector.tensor_tensor(out=p2s[:], in0=p2_all[:, fs], in1=r_bc,
                                        op=AOP.mult)
                e2 = fpool.tile([T, CH], f32, tag="e2")
                nc.scalar.activation(out=e2[:], in_=p2s[:], func=AF.Exp)
                den = fpool.tile([T, CH], f32, tag="den")
                nc.vector.tensor_tensor(out=den[:], in0=stats_all[0:T, fs],
                                        in1=e2[:], op=AOP.add)
                pe2 = fpool.tile([T, CH], f32, tag="pe2")
                nc.vector.tensor_tensor(out=pe2[:], in0=p2s[:], in1=e2[:],
                                        op=AOP.mult)
                num = fpool.tile([T, CH], f32, tag="num")
                nc.vector.tensor_tensor(out=num[:], in0=spe1[:], in1=pe2[:],
                                        op=AOP.add)
                rden = fpool.tile([T, CH], f32, tag="rden")
                nc.vector.reciprocal_approx_fast(out=rden[:], in_=den[:])
                sT = fpool.tile([T, CH], f32, tag="sT")
                nc.vector.tensor_tensor(out=sT[:], in0=num[:], in1=rden[:],
                                        op=AOP.mult)
                # sigmoid(s+b)*mask = (tanh((s+b)/2) + 1) * halfmask
                th = fpool.tile([T, CH], f32, tag="th")
                nc.scalar.activation(out=th[:], in_=sT[:], func=AF.Tanh,
                                     bias=bhalf[:], scale=0.5)
                nc.vector.scalar_tensor_tensor(
                    out=out_all[:, fs], in0=th[:], scalar=1.0, in1=hm_bc,
                    op0=AOP.add, op1=AOP.mult)

            # deferred per-group epilogue: exp/pe1/stats/evac/p2/finalize of
            # group g run interleaved into group g+1's PE stream so the PE
            # never stalls waiting for the ACT exp at a group boundary.
            def drain_prev(pv):
                gi_p = pv["gi"]
                nc.scalar.activation(
                    out=pv["e1pe1"][:, :, 0:T], in_=pv["p1q"][:], func=AF.Exp
                )
                nc.vector.tensor_tensor(
                    out=pv["e1pe1"][:, :, T:2 * T], in0=pv["p1q"][:],
                    in1=pv["e1pe1"][:, :, 0:T], op=AOP.mult,
                )
                for t in range(KT):
                    nc.tensor.matmul(
                        out=pv["acc"][0:17, P:2 * P], lhsT=pv["e1pe1"][:, t, :],
                        rhs=pv["oh"][:, t, :],
                        start=(t == 0), stop=(t == KT - 1),
                    )
                relu_aggT = fpool.tile([P, P], bf16, tag="relu_aggT")
                nc.scalar.activation(out=relu_aggT[:], in_=pv["acc"][:, 0:P],
                                     func=AF.Relu)
                nc.scalar.activation(out=stats_all[:, gi_p * P:(gi_p + 1) * P],
                                     in_=pv["acc"][0:17, P:2 * P], func=AF.Copy)
                p2T = ps_fin.tile([T, P], f32, tag="p2T")
                nc.tensor.matmul(
                    out=p2T[:], lhsT=wt[:], rhs=relu_aggT[:], start=True,
                    stop=True,
                )
                nc.scalar.activation(out=p2_all[:, gi_p * P:(gi_p + 1) * P],
                                     in_=p2T[:], func=AF.Copy)
                if gi_p < 16:
                    if (gi_p + 1) % CG == 0:
                        finalize_chunk(gi_p // CG * 512, 512)
                elif gi_p == 17:
                    finalize_chunk(16 * P, 2 * P)
                elif gi_p >= 18:
                    finalize_chunk(gi_p * P, P)

            prev = None
            pending_p1: list[tuple] = []  # (hTq_tile, p1q_tile, t)
            qrr = [0]  # global gather queue round-robin
            for gi in range(G):
                # ---- group loads ----
                edge_g = epool.tile([P, KT, D], bf16)
                nc.sync.dma_start(out=edge_g[:], in_=edge_d[:, gi, :, :])

                # one-hot S (host-precomputed relayout of dst; fp8 is
                # exact for 0/1 and halves the DMA bytes)
                oh_g = opool.tile([P, KT, P], fp8)
                nc.sync.dma_start(out=oh_g[:], in_=onehot_d[:, gi, :, :])

                gath_g = gpool.tile([P, KT, D], bf16)
                msg_g = mpool.tile([P, KT, D], bf16)
                # SWDGE descriptor ring fits <1024 descs per DMA: chunk to 7
                # tiles (896 gather descriptors) per dma_gather; msg = gath +
                # edge runs per chunk so PE work starts as soon as chunk 0
                # lands.
                for ci, c0 in enumerate(range(0, KT, 7)):
                    cn = min(7, KT - c0)
                    nc.gpsimd.dma_gather(
                        gath_g[:, c0:c0 + cn, :],
                        srcemb_d[:],
                        srcidx_all[:, gi * (L // 16) + c0 * 8:
                                   gi * (L // 16) + (c0 + cn) * 8],
                        cn * P,
                        cn * P,
                        D,
                        queue_num=qrr[0] % 4,
                    )
                    qrr[0] += 1
                    nc.vector.tensor_tensor(
                        out=msg_g[:, c0:c0 + cn, :], in0=gath_g[:, c0:c0 + cn, :],
                        in1=edge_g[:, c0:c0 + cn, :], op=AOP.add)

                # one PSUM tile for both accumulators (saves a bank each):
                # cols 0:P = aggT [d, n] sum(msg); cols P:2P rows 0:17 =
                # statsT (rows 0:8 sum(e1)T, 8:16 sum(p1*e1)T, 16 deg)
                acc = ps_acc.tile([P, 2 * P], f32, tag="acc")
                p1q = ps_p1.tile([P, KT, T], f32)
                e1pe1 = wpool.tile([P, KT, 2 * T + 1], bf16, tag="e1pe1")
                nc.vector.memset(e1pe1[:, :, 2 * T:2 * T + 1], 1.0)
                hTq = wpool.tile([P, KT, P], bf16, tag="hTq")
                # p1 matmuls for quad q are emitted during quad q+1 so the
                # ACT relu of quad q hides under quad q+1's PE streams.
                for h0 in range(0, KT, 4):
                    hn = min(4, KT - h0)
                    msgT = ps_msg.tile([P, hn, P], f32, tag="msgT")
                    for t in range(h0, h0 + hn):
                        # msgT[d, e] = msg.T (transpose via ident)
                        nc.tensor.matmul(
                            out=msgT[:, t - h0, :], lhsT=msg_g[:, t, :],
                            rhs=ident[:], start=True, stop=True,
                        )
                        # aggT[d, n] += msg.T @ S (shared lhsT with transpose)
                        nc.tensor.matmul(
                            out=acc[:, 0:P], lhsT=msg_g[:, t, :],
                            rhs=oh_g[:, t, :],
                            start=(t == 0), stop=(t == KT - 1),
                        )
                        if pending_p1:
                            hq, pq, tp = pending_p1.pop(0)
                            nc.tensor.matmul(
                                out=pq[:, tp, :], lhsT=hq[:, tp, :], rhs=wt[:],
                                start=True, stop=True,
                            )
                    if h0 == 0 and prev is not None:
                        drain_prev(prev)
                        prev = None
                    # hT = relu(msgT) per quad (ACT, PSUM->SBUF)
                    nc.scalar.activation(out=hTq[:, h0:h0 + hn, :],
                                         in_=msgT[:], func=AF.Relu)
                    pending_p1.extend(
                        (hTq, p1q, t) for t in range(h0, h0 + hn))
                prev = {"gi": gi, "oh": oh_g, "e1pe1": e1pe1, "p1q": p1q,
                        "acc": acc}

            for hq, pq, tp in pending_p1:
                nc.tensor.matmul(
                    out=pq[:, tp, :], lhsT=hq[:, tp, :], rhs=wt[:],
                    start=True, stop=True,
                )
            pending_p1.clear()
            drain_prev(prev)

            nc.sync.dma_start(out=out_d[:], in_=out_all[:])

    nc.compile()
    return nc


def _host_prep(src_embedding, edge_embedding, W, b, src, dst):
    """Sort/pad/shard edges on the host; returns (KT, in_maps)."""
    src = np.asarray(src).astype(np.int64)
    dst = np.asarray(dst).astype(np.int64)
    edge_embedding = np.asarray(edge_embedding, dtype=np.float32)
    src_embedding = np.asarray(src_embedding, dtype=np.float32)
    W = np.asarray(W, dtype=np.float32)
    b = np.asarray(b, dtype=np.float32)

    # ---- balance edge counts across the 160 (core, group) bins by
    # permuting the node->group assignment (LPT greedy on node in-degree):
    # KT is set by the heaviest group, so balancing trims ~6% of all
    # per-tile work including the pacing gather stream ----
    import heapq

    deg_n = np.bincount(dst, minlength=N_NODES)
    node_order = np.argsort(-deg_n, kind="stable")
    heap = [(0, g) for g in range(N_GROUPS)]
    heapq.heapify(heap)
    nslot = np.zeros(N_GROUPS, dtype=np.int64)
    g_of_node = np.empty(N_NODES, dtype=np.int64)
    slot_of_node = np.empty(N_NODES, dtype=np.int64)
    spill = []
    for n in node_order:
        load, g = heapq.heappop(heap)
        g_of_node[n] = g
        slot_of_node[n] = nslot[g]
        nslot[g] += 1
        item = (load + int(deg_n[n]), g)
        if nslot[g] < P:
            heapq.heappush(heap, item)
        else:
            spill.append(item)
        if not heap:
            heap = spill
            heapq.heapify(heap)
            spill = []
    node_of = np.full((N_GROUPS, P), -1, dtype=np.int64)
    node_of[g_of_node, slot_of_node] = np.arange(N_NODES)

    grp = g_of_node[dst]
    order = np.argsort(grp, kind="stable")
    s_src = src[order]
    s_dstslot = slot_of_node[dst[order]]
    s_edge = edge_embedding[order]

    counts = np.bincount(grp, minlength=N_GROUPS)
    KT = max(1, int(-(-counts.max() // P)))  # ceil / 128
    L = KT * P
    offs = np.concatenate([[0], np.cumsum(counts)])

    FP8 = ml_dtypes.float8_e4m3
    edge_c = np.zeros((CORES, P, G, KT, D), dtype=BF16)
    onehot_c = np.zeros((CORES, P, G, KT, P), dtype=FP8)
    srcidx_c = np.zeros((CORES, P, G * L // 16), dtype=np.int16)

    for g in range(N_GROUPS):
        c, gi = divmod(g, G)
        o0, o1 = int(offs[g]), int(offs[g + 1])
        cnt = o1 - o0
        if cnt:
            blk = np.zeros((L, D), dtype=BF16)
            blk[:cnt] = s_edge[o0:o1].astype(BF16)
            # edge t*128+p -> [p, t, :]
            edge_c[c, :, gi, :, :] = blk.reshape(KT, P, D).transpose(1, 0, 2)
            oh = np.zeros((L, P), dtype=FP8)
            oh[np.arange(cnt), s_dstslot[o0:o1]] = 1.0
            onehot_c[c, :, gi, :, :] = oh.reshape(KT, P, P).transpose(1, 0, 2)
        ids = np.zeros(L, dtype=np.int16)
        ids[:cnt] = s_src[o0:o1].astype(np.int16)
        # dma_gather index layout: logical i -> [i % 16, i // 16], x8 replicas
        wrapped = ids.reshape(L // 16, 16).T  # [16, L//16]
        srcidx_c[c, :, gi * (L // 16):(gi + 1) * (L // 16)] = np.tile(
            wrapped, (8, 1)
        )

    consts = {
        "srcemb": src_embedding.astype(BF16),
        "ident": np.eye(P, dtype=np.float32).astype(BF16),
        "wt": W.T.copy().astype(BF16),
        "ones18": np.ones((1, T), dtype=np.float32).astype(BF16),
        "bhalf": (b / 2.0).reshape(T, 1).astype(np.float32),
    }
    in_maps = [
        {
            "edge": edge_c[c],
            "onehot": onehot_c[c],
            "srcidx": srcidx_c[c],
            **consts,
        }
        for c in range(CORES)
    ]
    return KT, in_maps, node_of


def kernel(src_embedding, edge_embedding, W, b, src, dst):
    global LAST_RESULT
    KT, in_maps, node_of = _host_prep(src_embedding, edge_embedding, W, b,
                                      src, dst)
    run = _get_runner(KT)
    outs = run(in_maps)
    LAST_RESULT = None
    flat = np.empty((N_PAD, T), dtype=np.float32)
    for c in range(CORES):
        blk = np.asarray(outs[c]["out"], dtype=np.float32)
        flat[c * NODES_PER_CORE:(c + 1) * NODES_PER_CORE] = blk.T
    nodes = node_of.ravel()
    valid = nodes >= 0
    out = np.empty((N_NODES, T), dtype=np.float32)
    out[nodes[valid]] = flat[valid]
    return out


class _Runner:
    """Cached PJRT executor for a compiled Bass module (mirrors
    bass2jax.run_bass_via_pjrt but keeps the jitted callable + device inputs
    so repeated calls don't re-lower, and so timing loops are possible)."""

    def __init__(self, nc):
        import jax
        from jax.sharding import Mesh, PartitionSpec
        from jax.experimental.shard_map import shard_map
        import concourse.mybir as mybir
        from concourse import bass2jax

        bass2jax.install_neuronx_cc_hook()
        self.nc = nc
        in_names, out_names, out_avals, zero_outs = [], [], [], []
        for alloc in nc.m.functions[0].allocations:
            if not isinstance(alloc, mybir.MemoryLocationSet):
                continue
            name = alloc.memorylocations[0].name
            if alloc.kind == "ExternalInput":
                in_names.append(name)
            elif alloc.kind == "ExternalOutput":
                out_names.append(name)
                shape = tuple(alloc.tensor_shape)
                dtype = mybir.dt.np(alloc.dtype)
                out_avals.append(jax.core.ShapedArray(shape, dtype))
                zero_outs.append(np.zeros(shape, dtype))
        assert nc.partition_id_tensor is None, "partition id unused"
        self.in_names = list(in_names)
        self.out_names = out_names
        self.out_avals = out_avals
        self.zero_outs = zero_outs
        n_params = len(in_names)
        n_outs = len(out_avals)
        all_in_names = in_names + out_names
        donate = tuple(range(n_params, n_params + n_outs))

        def _body(*args):
            outs = bass2jax._bass_exec_p.bind(
                *args,
                out_avals=tuple(out_avals),
                in_names=tuple(all_in_names),
                out_names=tuple(out_names),
                lowering_input_output_aliases=(),
                sim_require_finite=True,
                sim_require_nnan=True,
                nc=nc,
            )
            return tuple(outs)

        devices = jax.devices()[:CORES]
        self.mesh = Mesh(np.asarray(devices), ("core",))
        in_specs = (PartitionSpec("core"),) * (n_params + n_outs)
        out_specs = (PartitionSpec("core"),) * n_outs
        self.fn = jax.jit(
            shard_map(_body, mesh=self.mesh, in_specs=in_specs,
                      out_specs=out_specs, check_rep=False),
            donate_argnums=donate, keep_unused=True,
        )
        self._dev_inputs = None

    def set_inputs(self, in_maps):
        import jax
        from jax.sharding import NamedSharding, PartitionSpec

        concat_in = [
            np.concatenate([np.asarray(in_maps[c][name]) for c in range(CORES)],
                           axis=0)
            for name in self.in_names
        ]
        sh = NamedSharding(self.mesh, PartitionSpec("core"))
        self._dev_inputs = [jax.device_put(a, sh) for a in concat_in]

    def execute(self):
        """One NEFF execution (inputs already on device). Returns jax arrays."""
        import jax
        from jax.sharding import NamedSharding, PartitionSpec

        sh = NamedSharding(self.mesh, PartitionSpec("core"))
        zeros = [
            jax.device_put(np.zeros((CORES * z.shape[0], *z.shape[1:]), z.dtype), sh)
            for z in self.zero_outs
        ]
        out = self.fn(*self._dev_inputs, *zeros)
        jax.block_until_ready(out)
        return out

    def __call__(self, in_maps):
        self.set_inputs(in_maps)
        out_arrs = self.execute()
        return [
            {
                name: np.asarray(out_arrs[i]).reshape(
                    CORES, *self.out_avals[i].shape)[c]
                for i, name in enumerate(self.out_names)
            }
            for c in range(CORES)
        ]


def _get_runner(KT: int) -> _Runner:
    run = _BUILD_CACHE.get(KT)
    if run is None:
        run = _Runner(_build(KT))
        _BUILD_CACHE[KT] = run
    return run


# revision 28
# speedup vs baseline: 1.0461x; 1.0461x over previous
"""Trainium2 Bass kernel for nn_CETLayer (GNN message passing + per-node softmax).

Strategy (8 NeuronCores, no collectives needed):
  - Shard edges BY DST RANGE: core c owns nodes [c*2560, (c+1)*2560) and all
    edges pointing into that range. Each core computes its output slice fully
    locally; host concatenates.
  - Host prep: sort edges by dst, group into 128-node "groups" (20 per core),
    pad each group's edge list to KT*128 (KT = global max) so all cores run
    the identical (SPMD) program. The per-edge dst one-hot matrices S
    ([128 edges, 128 nodes] bf16 per tile) are precomputed host-side (a pure
    relayout of the dst index tensor) and DMA'd in, rather than generated
    on-device (the DVE is_equal path measured ~290ns/tile plus long
    pool-reuse stalls and was the top engine).
  - Device per edge-tile [128 edges]:
      gather src rows (gpsimd dma_gather, bf16),
      msg = gath + edge on DVE (bf16 2x mode, whole group in one op),
      one PE transpose-matmul (lhsT=msg, rhs=ident) -> msgT PSUM,
      one PE agg matmul (lhsT=msg again - shared stationary, rhs=S) -> aggT,
      relu (ACT, per 4-tile quad) -> hT, p1 = hT.T @ W.T (PE, emitted one
      quad late so the ACT relu hides under the next quad's matmuls),
      e1 = exp(p1) (ACT, group-level), pe1 = p1*e1 (DVE),
      stats matmul (lhsT=e1pe1 [e,17] - 17-col stationary, rhs=S) -> statsT
      accumulating sum(e1), sum(p1*e1), deg over the group's tiles.
  - Softmax max-subtraction is skipped (mathematically identical, values are
    O(1) so exp() is safe in f32), and the fc bias b is algebraically pulled
    out of the softmax (weights are shift-invariant; sum(w)=1 puts +b back
    at the end): s = (sum(p1'*e1') + p2'*e2')/(sum(e1')+e2') + b.
  - Group finalize computes p2 path, softmax combine, sigmoid (via tanh,
    which shares the ACT table with exp/relu/copy), deg>0 mask. Divisions
    use reciprocal_approx_fast (~18 bits, 5x faster than DVE reciprocal).

kernel(**inputs) takes the FULL inputs and returns the FULL [20000, 8] f32
output. Compute runs in bf16 on the PE (f32 PSUM accumulation).
"""

import dataclasses

import numpy as np
import ml_dtypes

import concourse.bacc as bacc
import concourse.mybir as mybir
import concourse.tile as tile

P = 128
N_NODES = 20000
D = 128
T = 8
CORES = 8
G = 20                      # groups (of 128 nodes) per core
NODES_PER_CORE = G * P      # 2560
N_PAD = CORES * NODES_PER_CORE  # 20480
N_GROUPS = CORES * G        # 160
PAD_DST = 384.0             # out of [0,128) -> zero one-hot row (exact in bf16)

BF16 = ml_dtypes.bfloat16

_BUILD_CACHE: dict[int, object] = {}
LAST_RESULT = None  # BassKernelResults of the most recent run (for test.py)


def _build(KT: int):
    """Build + compile the SPMD program for KT edge-tiles per node-group."""
    L = KT * P  # padded edges per group
    f32 = mybir.dt.float32
    bf16 = mybir.dt.bfloat16
    i16 = mybir.dt.int16
    AOP = mybir.AluOpType
    AF = mybir.ActivationFunctionType

    nc = bacc.Bacc("TRN2", target_bir_lowering=False, enable_partition_id=False,
                   num_swdge_queues=4)

    fp8 = mybir.dt.float8e4
    edge_d = nc.dram_tensor("edge", [P, G, KT, D], bf16, kind="ExternalInput")
    onehot_d = nc.dram_tensor("onehot", [P, G, KT, P], bf16, kind="ExternalInput")
    srcidx_d = nc.dram_tensor("srcidx", [P, G * L // 16], i16, kind="ExternalInput")
    srcemb_d = nc.dram_tensor("srcemb", [N_NODES, D], bf16, kind="ExternalInput")
    ident_d = nc.dram_tensor("ident", [P, P], bf16, kind="ExternalInput")
    wt_d = nc.dram_tensor("wt", [D, T], bf16, kind="ExternalInput")
    ones18_d = nc.dram_tensor("ones18", [1, T], bf16, kind="ExternalInput")
    bhalf_d = nc.dram_tensor("bhalf", [T, 1], f32, kind="ExternalInput")
    out_d = nc.dram_tensor("out", [T, G * P], f32, kind="ExternalOutput")

    with tile.TileContext(nc) as tc, nc.allow_low_precision(reason="bf16 compute"):
        with (
            tc.tile_pool(name="const", bufs=1) as cpool,
            tc.tile_pool(name="edges", bufs=2) as epool,
            tc.tile_pool(name="onehot", bufs=2) as opool,
            tc.tile_pool(name="gath", bufs=2) as gpool,
            tc.tile_pool(name="msg", bufs=2) as mpool,
            tc.tile_pool(name="work", bufs=2) as wpool,
            tc.tile_pool(name="fin", bufs=2) as fpool,
            tc.tile_pool(name="ps_msg", bufs=2, space="PSUM") as ps_msg,
            tc.tile_pool(name="ps_p1", bufs=2, space="PSUM") as ps_p1,
            tc.tile_pool(name="ps_acc", bufs=2, space="PSUM") as ps_acc,
            tc.tile_pool(name="ps_fin", bufs=1, space="PSUM") as ps_fin,
        ):
            # ---- gather indices first: the Q7 gather stream paces the whole
            # kernel, so its inputs must land before anything else ----
            srcidx_all = cpool.tile([P, G * L // 16], i16)
            nc.sync.dma_start(out=srcidx_all[:, 0:L // 16],
                              in_=srcidx_d[:, 0:L // 16])
            nc.sync.dma_start(out=srcidx_all[:, L // 16:],
                              in_=srcidx_d[:, L // 16:])
            # ---- constants / whole-run SBUF residents ----
            ident = cpool.tile([P, P], bf16)
            nc.sync.dma_start(out=ident[:], in_=ident_d[:])
            wt = cpool.tile([D, T], bf16)
            nc.sync.dma_start(out=wt[:], in_=wt_d[:])
            ones18 = cpool.tile([1, T], bf16)
            nc.sync.dma_start(out=ones18[:], in_=ones18_d[:])
            bhalf = cpool.tile([T, 1], f32)
            nc.sync.dma_start(out=bhalf[:], in_=bhalf_d[:])
            out_all = cpool.tile([T, G * P], f32)
            # batched-finalize accumulators (whole run)
            NW = G * P  # 2560 nodes per core
            stats_all = cpool.tile([17, NW], f32)
            p2_all = cpool.tile([T, NW], f32)

            CH = 512
            CG = CH // P  # groups per finalize chunk

            def finalize_chunk(f0, CH):
                fs = slice(f0, f0 + CH)
                # partition-shifting moves must go through (SBUF->SBUF) DMA
                spe1 = fpool.tile([T, CH], f32, tag="spe1")
                nc.sync.dma_start(out=spe1[:], in_=stats_all[T:2 * T, fs])
                degrow = fpool.tile([1, CH], f32, tag="degrow")
                nc.sync.dma_start(out=degrow[:], in_=stats_all[16:17, fs])
                # r = 1/max(deg,1) ; halfmask = 0.5*(deg > 0)
                dmax = fpool.tile([1, CH], f32, tag="dmax")
                nc.vector.tensor_scalar(out=dmax[:], in0=degrow[:],
                                        scalar1=1.0, scalar2=None, op0=AOP.max)
                rm = fpool.tile([1, 2 * CH], f32, tag="rm")
                nc.vector.reciprocal_approx_fast(out=rm[:, 0:CH], in_=dmax[:])
                nc.vector.tensor_scalar(out=rm[:, CH:2 * CH], in0=degrow[:],
                                        scalar1=0.0, scalar2=0.5,
                                        op0=AOP.is_gt, op1=AOP.mult)
                # broadcast r|halfmask to the 8 type-partitions via PE
                rm_bf = fpool.tile([1, 2 * CH], bf16, tag="rm_bf")
                nc.vector.tensor_scalar(out=rm_bf[:], in0=rm[:], scalar1=0.0,
                                        scalar2=None, op0=AOP.add)
                bc_sb = fpool.tile([T, 2 * CH], f32, tag="bc_sb")
                for c0 in (0, CH):
                    bc = ps_fin.tile([T, CH], f32, tag="p2T")
                    nc.tensor.matmul(out=bc[:], lhsT=ones18[:],
                                     rhs=rm_bf[:, c0:c0 + CH], start=True, stop=True)
                    nc.scalar.activation(out=bc_sb[:, c0:c0 + CH], in_=bc[:],
                                         func=AF.Copy)
                r_bc = bc_sb[:, 0:CH]
                hm_bc = bc_sb[:, CH:2 * CH]
                p2s = fpool.tile([T, CH], f32, tag="p2s")
                nc.vector.tensor_tensor(out=p2s[:], in0=p2_all[:, fs], in1=r_bc,
                                        op=AOP.mult)
                e2 = fpool.tile([T, CH], f32, tag="e2")
                nc.scalar.activation(out=e2[:], in_=p2s[:], func=AF.Exp)
                den = fpool.tile([T, CH], f32, tag="den")
                nc.vector.tensor_tensor(out=den[:], in0=stats_all[0:T, fs],
                                        in1=e2[:], op=AOP.add)
                pe2 = fpool.tile([T, CH], f32, tag="pe2")
                nc.vector.tensor_tensor(out=pe2[:], in0=p2s[:], in1=e2[:],
                                        op=AOP.mult)
                num = fpool.tile([T, CH], f32, tag="num")
                nc.vector.tensor_tensor(out=num[:], in0=spe1[:], in1=pe2[:],
                                        op=AOP.add)
                rden = fpool.tile([T, CH], f32, tag="rden")
                nc.vector.reciprocal_approx_fast(out=rden[:], in_=den[:])
                sT = fpool.tile([T, CH], f32, tag="sT")
                nc.vector.tensor_tensor(out=sT[:], in0=num[:], in1=rden[:],
                                        op=AOP.mult)
                # sigmoid(s+b)*mask = (tanh((s+b)/2) + 1) * halfmask
                th = fpool.tile([T, CH], f32, tag="th")
                nc.scalar.activation(out=th[:], in_=sT[:], func=AF.Tanh,
                                     bias=bhalf[:], scale=0.5)
                nc.vector.scalar_tensor_tensor(
                    out=out_all[:, fs], in0=th[:], scalar=1.0, in1=hm_bc,
                    op0=AOP.add, op1=AOP.mult)

            # deferred per-group epilogue: exp/pe1/stats/evac/p2/finalize of
            # group g run interleaved into group g+1's PE stream so the PE
            # never stalls waiting for the ACT exp at a group boundary.
            def drain_prev(pv):
                gi_p = pv["gi"]
                nc.scalar.activation(
                    out=pv["e1pe1"][:, :, 0:T], in_=pv["p1q"][:], func=AF.Exp
                )
                nc.vector.tensor_tensor(
                    out=pv["e1pe1"][:, :, T:2 * T], in0=pv["p1q"][:],
                    in1=pv["e1pe1"][:, :, 0:T], op=AOP.mult,
                )
                for t in range(KT):
                    nc.tensor.matmul(
                        out=pv["acc"][0:17, P:2 * P], lhsT=pv["e1pe1"][:, t, :],
                        rhs=pv["oh"][:, t, :],
                        start=(t == 0), stop=(t == KT - 1),
                    )
                relu_aggT = fpool.tile([P, P], bf16, tag="relu_aggT")
                nc.scalar.activation(out=relu_aggT[:], in_=pv["acc"][:, 0:P],
                                     func=AF.Relu)
                nc.scalar.activation(out=stats_all[:, gi_p * P:(gi_p + 1) * P],
                                     in_=pv["acc"][0:17, P:2 * P], func=AF.Copy)
                p2T = ps_fin.tile([T, P], f32, tag="p2T")
                nc.tensor.matmul(
                    out=p2T[:], lhsT=wt[:], rhs=relu_aggT[:], start=True,
                    stop=True,
                )
                nc.scalar.activation(out=p2_all[:, gi_p * P:(gi_p + 1) * P],
                                     in_=p2T[:], func=AF.Copy)
                if gi_p < 16:
                    if (gi_p + 1) % CG == 0:
                        finalize_chunk(gi_p // CG * 512, 512)
                elif gi_p == 17:
                    finalize_chunk(16 * P, 2 * P)
                elif gi_p >= 18:
                    finalize_chunk(gi_p * P, P)

            prev = None
            pending_p1: list[tuple] = []  # (hTq_tile, p1q_tile, t)
            qrr = [0]  # global gather queue round-robin
            for gi in range(G):
                # ---- group loads ----
                edge_g = epool.tile([P, KT, D], bf16)
                nc.sync.dma_start(out=edge_g[:], in_=edge_d[:, gi, :, :])

                # one-hot S (host-precomputed relayout of dst)
                oh_g = opool.tile([P, KT, P], bf16)
                nc.sync.dma_start(out=oh_g[:], in_=onehot_d[:, gi, :, :])

                gath_g = gpool.tile([P, KT, D], bf16)
                msg_g = mpool.tile([P, KT, D], bf16)
                # SWDGE descriptor ring fits <1024 descs per DMA: chunk to 7
                # tiles (896 gather descriptors) per dma_gather; msg = gath +
                # edge runs per chunk so PE work starts as soon as chunk 0
                # lands.
                for ci, c0 in enumerate(range(0, KT, 7)):
                    cn = min(7, KT - c0)
                    nc.gpsimd.dma_gather(
                        gath_g[:, c0:c0 + cn, :],
                        srcemb_d[:],
                        srcidx_all[:, gi * (L // 16) + c0 * 8:
                                   gi * (L // 16) + (c0 + cn) * 8],
                        cn * P,
                        cn * P,
                        D,
                        queue_num=qrr[0] % 4,
                    )
                    qrr[0] += 1
                    nc.vector.tensor_tensor(
                        out=msg_g[:, c0:c0 + cn, :], in0=gath_g[:, c0:c0 + cn, :],
                        in1=edge_g[:, c0:c0 + cn, :], op=AOP.add)

                # one PSUM tile for both accumulators (saves a bank each):
                # cols 0:P = aggT [d, n] sum(msg); cols P:2P rows 0:17 =
                # statsT (rows 0:8 sum(e1)T, 8:16 sum(p1*e1)T, 16 deg)
                acc = ps_acc.tile([P, 2 * P], f32, tag="acc")
                p1q = ps_p1.tile([P, KT, T], f32)
                e1pe1 = wpool.tile([P, KT, 2 * T + 1], bf16, tag="e1pe1")
                nc.vector.memset(e1pe1[:, :, 2 * T:2 * T + 1], 1.0)
                hTq = wpool.tile([P, KT, P], bf16, tag="hTq")
                # p1 matmuls for quad q are emitted during quad q+1 so the
                # ACT relu of quad q hides under quad q+1's PE streams.
                for h0 in range(0, KT, 4):
                    hn = min(4, KT - h0)
                    msgT = ps_msg.tile([P, hn, P], f32, tag="msgT")
                    for t in range(h0, h0 + hn):
                        # msgT[d, e] = msg.T (transpose via ident)
                        nc.tensor.matmul(
                            out=msgT[:, t - h0, :], lhsT=msg_g[:, t, :],
                            rhs=ident[:], start=True, stop=True,
                        )
                        # aggT[d, n] += msg.T @ S (shared lhsT with transpose)
                        nc.tensor.matmul(
                            out=acc[:, 0:P], lhsT=msg_g[:, t, :],
                            rhs=oh_g[:, t, :],
                            start=(t == 0), stop=(t == KT - 1),
                        )
                        if pending_p1:
                            hq, pq, tp = pending_p1.pop(0)
                            nc.tensor.matmul(
                                out=pq[:, tp, :], lhsT=hq[:, tp, :], rhs=wt[:],
                                start=True, stop=True,
                            )
                    if h0 == 0 and prev is not None:
                        drain_prev(prev)
                        prev = None
                    # hT = relu(msgT) per quad (ACT, PSUM->SBUF)
                    nc.scalar.activation(out=hTq[:, h0:h0 + hn, :],
                                         in_=msgT[:], func=AF.Relu)
                    pending_p1.extend(
                        (hTq, p1q, t) for t in range(h0, h0 + hn))
                prev = {"gi": gi, "oh": oh_g, "e1pe1": e1pe1, "p1q": p1q,
                        "acc": acc}

            for hq, pq, tp in pending_p1:
                nc.tensor.matmul(
                    out=pq[:, tp, :], lhsT=hq[:, tp, :], rhs=wt[:],
                    start=True, stop=True,
                )
            pending_p1.clear()
            drain_prev(prev)

            nc.sync.dma_start(out=out_d[:], in_=out_all[:])

    nc.compile()
    return nc


def _host_prep(src_embedding, edge_embedding, W, b, src, dst):
    """Sort/pad/shard edges on the host; returns (KT, in_maps)."""
    src = np.asarray(src).astype(np.int64)
    dst = np.asarray(dst).astype(np.int64)
    edge_embedding = np.asarray(edge_embedding, dtype=np.float32)
    src_embedding = np.asarray(src_embedding, dtype=np.float32)
    W = np.asarray(W, dtype=np.float32)
    b = np.asarray(b, dtype=np.float32)

    # ---- balance edge counts across the 160 (core, group) bins by
    # permuting the node->group assignment (LPT greedy on node in-degree):
    # KT is set by the heaviest group, so balancing trims ~6% of all
    # per-tile work including the pacing gather stream ----
    import heapq

    deg_n = np.bincount(dst, minlength=N_NODES)
    node_order = np.argsort(-deg_n, kind="stable")
    heap = [(0, g) for g in range(N_GROUPS)]
    heapq.heapify(heap)
    nslot = np.zeros(N_GROUPS, dtype=np.int64)
    g_of_node = np.empty(N_NODES, dtype=np.int64)
    slot_of_node = np.empty(N_NODES, dtype=np.int64)
    spill = []
    for n in node_order:
        load, g = heapq.heappop(heap)
        g_of_node[n] = g
        slot_of_node[n] = nslot[g]
        nslot[g] += 1
        item = (load + int(deg_n[n]), g)
        if nslot[g] < P:
            heapq.heappush(heap, item)
        else:
            spill.append(item)
        if not heap:
            heap = spill
            heapq.heapify(heap)
            spill = []
    node_of = np.full((N_GROUPS, P), -1, dtype=np.int64)
    node_of[g_of_node, slot_of_node] = np.arange(N_NODES)

    grp = g_of_node[dst]
    order = np.argsort(grp, kind="stable")
    s_src = src[order]
    s_dstslot = slot_of_node[dst[order]]
    s_edge = edge_embedding[order]

    counts = np.bincount(grp, minlength=N_GROUPS)
    KT = max(1, int(-(-counts.max() // P)))  # ceil / 128
    L = KT * P
    offs = np.concatenate([[0], np.cumsum(counts)])

    edge_c = np.zeros((CORES, P, G, KT, D), dtype=BF16)
    onehot_c = np.zeros((CORES, P, G, KT, P), dtype=BF16)
    srcidx_c = np.zeros((CORES, P, G * L // 16), dtype=np.int16)

    for g in range(N_GROUPS):
        c, gi = divmod(g, G)
        o0, o1 = int(offs[g]), int(offs[g + 1])
        cnt = o1 - o0
        if cnt:
            blk = np.zeros((L, D), dtype=BF16)
            blk[:cnt] = s_edge[o0:o1].astype(BF16)
            # edge t*128+p -> [p, t, :]
            edge_c[c, :, gi, :, :] = blk.reshape(KT, P, D).transpose(1, 0, 2)
            oh = np.zeros((L, P), dtype=BF16)
            oh[np.arange(cnt), s_dstslot[o0:o1]] = 1.0
            onehot_c[c, :, gi, :, :] = oh.reshape(KT, P, P).transpose(1, 0, 2)
        ids = np.zeros(L, dtype=np.int16)
        ids[:cnt] = s_src[o0:o1].astype(np.int16)
        # dma_gather index layout: logical i -> [i % 16, i // 16], x8 replicas
        wrapped = ids.reshape(L // 16, 16).T  # [16, L//16]
        srcidx_c[c, :, gi * (L // 16):(gi + 1) * (L // 16)] = np.tile(
            wrapped, (8, 1)
        )

    consts = {
        "srcemb": src_embedding.astype(BF16),
        "ident": np.eye(P, dtype=np.float32).astype(BF16),
        "wt": W.T.copy().astype(BF16),
        "ones18": np.ones((1, T), dtype=np.float32).astype(BF16),
        "bhalf": (b / 2.0).reshape(T, 1).astype(np.float32),
    }
    in_maps = [
        {
            "edge": edge_c[c],
            "onehot": onehot_c[c],
            "srcidx": srcidx_c[c],
            **consts,
        }
        for c in range(CORES)
    ]
    return KT, in_maps, node_of


def kernel(src_embedding, edge_embedding, W, b, src, dst):
    global LAST_RESULT
    KT, in_maps, node_of = _host_prep(src_embedding, edge_embedding, W, b,
                                      src, dst)
    run = _get_runner(KT)
    outs = run(in_maps)
    LAST_RESULT = None
    flat = np.empty((N_PAD, T), dtype=np.float32)
    for c in range(CORES):
        blk = np.asarray(outs[c]["out"], dtype=np.float32)
        flat[c * NODES_PER_CORE:(c + 1) * NODES_PER_CORE] = blk.T
    nodes = node_of.ravel()
    valid = nodes >= 0
    out = np.empty((N_NODES, T), dtype=np.float32)
    out[nodes[valid]] = flat[valid]
    return out


class _Runner:
    """Cached PJRT executor for a compiled Bass module (mirrors
    bass2jax.run_bass_via_pjrt but keeps the jitted callable + device inputs
    so repeated calls don't re-lower, and so timing loops are possible)."""

    def __init__(self, nc):
        import jax
        from jax.sharding import Mesh, PartitionSpec
        from jax.experimental.shard_map import shard_map
        import concourse.mybir as mybir
        from concourse import bass2jax

        bass2jax.install_neuronx_cc_hook()
        self.nc = nc
        in_names, out_names, out_avals, zero_outs = [], [], [], []
        for alloc in nc.m.functions[0].allocations:
            if not isinstance(alloc, mybir.MemoryLocationSet):
                continue
            name = alloc.memorylocations[0].name
            if alloc.kind == "ExternalInput":
                in_names.append(name)
            elif alloc.kind == "ExternalOutput":
                out_names.append(name)
                shape = tuple(alloc.tensor_shape)
                dtype = mybir.dt.np(alloc.dtype)
                out_avals.append(jax.core.ShapedArray(shape, dtype))
                zero_outs.append(np.zeros(shape, dtype))
        assert nc.partition_id_tensor is None, "partition id unused"
        self.in_names = list(in_names)
        self.out_names = out_names
        self.out_avals = out_avals
        self.zero_outs = zero_outs
        n_params = len(in_names)
        n_outs = len(out_avals)
        all_in_names = in_names + out_names
        donate = tuple(range(n_params, n_params + n_outs))

        def _body(*args):
            outs = bass2jax._bass_exec_p.bind(
                *args,
                out_avals=tuple(out_avals),
                in_names=tuple(all_in_names),
                out_names=tuple(out_names),
                lowering_input_output_aliases=(),
                sim_require_finite=True,
                sim_require_nnan=True,
                nc=nc,
            )
            return tuple(outs)

        devices = jax.devices()[:CORES]
        self.mesh = Mesh(np.asarray(devices), ("core",))
        in_specs = (PartitionSpec("core"),) * (n_params + n_outs)
        out_specs = (PartitionSpec("core"),) * n_outs
        self.fn = jax.jit(
            shard_map(_body, mesh=self.mesh, in_specs=in_specs,
                      out_specs=out_specs, check_rep=False),
            donate_argnums=donate, keep_unused=True,
        )
        self._dev_inputs = None

    def set_inputs(self, in_maps):
        import jax
        from jax.sharding import NamedSharding, PartitionSpec

        concat_in = [
            np.concatenate([np.asarray(in_maps[c][name]) for c in range(CORES)],
                           axis=0)
            for name in self.in_names
        ]
        sh = NamedSharding(self.mesh, PartitionSpec("core"))
        self._dev_inputs = [jax.device_put(a, sh) for a in concat_in]

    def execute(self):
        """One NEFF execution (inputs already on device). Returns jax arrays."""
        import jax
        from jax.sharding import NamedSharding, PartitionSpec

        sh = NamedSharding(self.mesh, PartitionSpec("core"))
        zeros = [
            jax.device_put(np.zeros((CORES * z.shape[0], *z.shape[1:]), z.dtype), sh)
            for z in self.zero_outs
        ]
        out = self.fn(*self._dev_inputs, *zeros)
        jax.block_until_ready(out)
        return out

    def __call__(self, in_maps):
        self.set_inputs(in_maps)
        out_arrs = self.execute()
        return [
            {
                name: np.asarray(out_arrs[i]).reshape(
                    CORES, *self.out_avals[i].shape)[c]
                for i, name in enumerate(self.out_names)
            }
            for c in range(CORES)
        ]


def _get_runner(KT: int) -> _Runner:
    run = _BUILD_CACHE.get(KT)
    if run is None:
        run = _Runner(_build(KT))
        _BUILD_CACHE[KT] = run
    return run


# revision 29
# speedup vs baseline: 1.1183x; 1.0691x over previous
"""Trainium2 Bass kernel for nn_CETLayer (GNN message passing + per-node softmax).

Strategy (8 NeuronCores, no collectives needed):
  - Shard edges BY DST RANGE: core c owns nodes [c*2560, (c+1)*2560) and all
    edges pointing into that range. Each core computes its output slice fully
    locally; host concatenates.
  - Host prep: sort edges by dst, group into 128-node "groups" (20 per core),
    pad each group's edge list to KT*128 (KT = global max) so all cores run
    the identical (SPMD) program. The per-edge dst one-hot matrices S
    ([128 edges, 128 nodes] bf16 per tile) are precomputed host-side (a pure
    relayout of the dst index tensor) and DMA'd in, rather than generated
    on-device (the DVE is_equal path measured ~290ns/tile plus long
    pool-reuse stalls and was the top engine).
  - Device per edge-tile [128 edges]:
      gather src rows (gpsimd dma_gather, bf16),
      msg = gath + edge on DVE (bf16 2x mode, whole group in one op),
      one PE transpose-matmul (lhsT=msg, rhs=ident) -> msgT PSUM,
      one PE agg matmul (lhsT=msg again - shared stationary, rhs=S) -> aggT,
      relu (ACT, per 4-tile quad) -> hT, p1 = hT.T @ W.T (PE, emitted one
      quad late so the ACT relu hides under the next quad's matmuls),
      e1 = exp(p1) (ACT, group-level), pe1 = p1*e1 (DVE),
      stats matmul (lhsT=e1pe1 [e,17] - 17-col stationary, rhs=S) -> statsT
      accumulating sum(e1), sum(p1*e1), deg over the group's tiles.
  - Softmax max-subtraction is skipped (mathematically identical, values are
    O(1) so exp() is safe in f32), and the fc bias b is algebraically pulled
    out of the softmax (weights are shift-invariant; sum(w)=1 puts +b back
    at the end): s = (sum(p1'*e1') + p2'*e2')/(sum(e1')+e2') + b.
  - Group finalize computes p2 path, softmax combine, sigmoid (via tanh,
    which shares the ACT table with exp/relu/copy), deg>0 mask. Divisions
    use reciprocal_approx_fast (~18 bits, 5x faster than DVE reciprocal).

kernel(**inputs) takes the FULL inputs and returns the FULL [20000, 8] f32
output. Compute runs in bf16 on the PE (f32 PSUM accumulation).
"""

import dataclasses

import numpy as np
import ml_dtypes

import concourse.bacc as bacc
import concourse.mybir as mybir
import concourse.tile as tile

P = 128
N_NODES = 20000
D = 128
T = 8
CORES = 8
G = 20                      # groups (of 128 nodes) per core
NODES_PER_CORE = G * P      # 2560
N_PAD = CORES * NODES_PER_CORE  # 20480
N_GROUPS = CORES * G        # 160
PAD_DST = 384.0             # out of [0,128) -> zero one-hot row (exact in bf16)

BF16 = ml_dtypes.bfloat16

_BUILD_CACHE: dict[int, object] = {}
LAST_RESULT = None  # BassKernelResults of the most recent run (for test.py)


def _build(KT: int):
    """Build + compile the SPMD program for KT edge-tiles per node-group."""
    L = KT * P  # padded edges per group
    f32 = mybir.dt.float32
    bf16 = mybir.dt.bfloat16
    i16 = mybir.dt.int16
    AOP = mybir.AluOpType
    AF = mybir.ActivationFunctionType

    nc = bacc.Bacc("TRN2", target_bir_lowering=False, enable_partition_id=False,
                   num_swdge_queues=4)

    edge_d = nc.dram_tensor("edge", [P, G, KT, D], bf16, kind="ExternalInput")
    dstloc_d = nc.dram_tensor("dstloc", [P, G * KT], bf16, kind="ExternalInput")
    srcidx_d = nc.dram_tensor("srcidx", [P, G * L // 16], i16, kind="ExternalInput")
    srcemb_d = nc.dram_tensor("srcemb", [N_NODES, D], bf16, kind="ExternalInput")
    ident_d = nc.dram_tensor("ident", [P, P], bf16, kind="ExternalInput")
    iota_d = nc.dram_tensor("iota", [P, P], bf16, kind="ExternalInput")
    wt_d = nc.dram_tensor("wt", [D, T], bf16, kind="ExternalInput")
    ones18_d = nc.dram_tensor("ones18", [1, T], bf16, kind="ExternalInput")
    bhalf_d = nc.dram_tensor("bhalf", [T, 1], f32, kind="ExternalInput")
    out_d = nc.dram_tensor("out", [T, G * P], f32, kind="ExternalOutput")

    with tile.TileContext(nc) as tc, nc.allow_low_precision(reason="bf16 compute"):
        with (
            tc.tile_pool(name="const", bufs=1) as cpool,
            tc.tile_pool(name="edges", bufs=2) as epool,
            tc.tile_pool(name="onehot", bufs=2) as opool,
            tc.tile_pool(name="gath", bufs=2) as gpool,
            tc.tile_pool(name="msg", bufs=2) as mpool,
            tc.tile_pool(name="work", bufs=2) as wpool,
            tc.tile_pool(name="fin", bufs=2) as fpool,
            tc.tile_pool(name="ps_msg", bufs=2, space="PSUM") as ps_msg,
            tc.tile_pool(name="ps_p1", bufs=2, space="PSUM") as ps_p1,
            tc.tile_pool(name="ps_acc", bufs=2, space="PSUM") as ps_acc,
            tc.tile_pool(name="ps_fin", bufs=1, space="PSUM") as ps_fin,
        ):
            # ---- gather indices first: the Q7 gather stream paces the whole
            # kernel, so its inputs must land before anything else ----
            srcidx_all = cpool.tile([P, G * L // 16], i16)
            nc.sync.dma_start(out=srcidx_all[:, 0:L // 16],
                              in_=srcidx_d[:, 0:L // 16])
            nc.sync.dma_start(out=srcidx_all[:, L // 16:],
                              in_=srcidx_d[:, L // 16:])
            # ---- constants / whole-run SBUF residents ----
            ident = cpool.tile([P, P], bf16)
            nc.sync.dma_start(out=ident[:], in_=ident_d[:])
            iota = cpool.tile([P, P], bf16)
            nc.sync.dma_start(out=iota[:], in_=iota_d[:])
            dstloc_all = cpool.tile([P, G * KT], bf16)
            nc.sync.dma_start(out=dstloc_all[:], in_=dstloc_d[:])
            wt = cpool.tile([D, T], bf16)
            nc.sync.dma_start(out=wt[:], in_=wt_d[:])
            ones18 = cpool.tile([1, T], bf16)
            nc.sync.dma_start(out=ones18[:], in_=ones18_d[:])
            bhalf = cpool.tile([T, 1], f32)
            nc.sync.dma_start(out=bhalf[:], in_=bhalf_d[:])
            out_all = cpool.tile([T, G * P], f32)
            # batched-finalize accumulators (whole run)
            NW = G * P  # 2560 nodes per core
            stats_all = cpool.tile([17, NW], f32)
            p2_all = cpool.tile([T, NW], f32)

            CH = 512
            CG = CH // P  # groups per finalize chunk

            def finalize_chunk(f0, CH):
                fs = slice(f0, f0 + CH)
                # partition-shifting moves must go through (SBUF->SBUF) DMA
                spe1 = fpool.tile([T, CH], f32, tag="spe1")
                nc.sync.dma_start(out=spe1[:], in_=stats_all[T:2 * T, fs])
                degrow = fpool.tile([1, CH], f32, tag="degrow")
                nc.sync.dma_start(out=degrow[:], in_=stats_all[16:17, fs])
                # r = 1/max(deg,1) ; halfmask = 0.5*(deg > 0)
                dmax = fpool.tile([1, CH], f32, tag="dmax")
                nc.vector.tensor_scalar(out=dmax[:], in0=degrow[:],
                                        scalar1=1.0, scalar2=None, op0=AOP.max)
                rm = fpool.tile([1, 2 * CH], f32, tag="rm")
                nc.vector.reciprocal_approx_fast(out=rm[:, 0:CH], in_=dmax[:])
                nc.vector.tensor_scalar(out=rm[:, CH:2 * CH], in0=degrow[:],
                                        scalar1=0.0, scalar2=0.5,
                                        op0=AOP.is_gt, op1=AOP.mult)
                # broadcast r|halfmask to the 8 type-partitions via PE
                rm_bf = fpool.tile([1, 2 * CH], bf16, tag="rm_bf")
                nc.vector.tensor_scalar(out=rm_bf[:], in0=rm[:], scalar1=0.0,
                                        scalar2=None, op0=AOP.add)
                bc_sb = fpool.tile([T, 2 * CH], f32, tag="bc_sb")
                for c0 in (0, CH):
                    bc = ps_fin.tile([T, CH], f32, tag="p2T")
                    nc.tensor.matmul(out=bc[:], lhsT=ones18[:],
                                     rhs=rm_bf[:, c0:c0 + CH], start=True, stop=True)
                    nc.scalar.activation(out=bc_sb[:, c0:c0 + CH], in_=bc[:],
                                         func=AF.Copy)
                r_bc = bc_sb[:, 0:CH]
                hm_bc = bc_sb[:, CH:2 * CH]
                p2s = fpool.tile([T, CH], f32, tag="p2s")
                nc.vector.tensor_tensor(out=p2s[:], in0=p2_all[:, fs], in1=r_bc,
                                        op=AOP.mult)
                e2 = fpool.tile([T, CH], f32, tag="e2")
                nc.scalar.activation(out=e2[:], in_=p2s[:], func=AF.Exp)
                den = fpool.tile([T, CH], f32, tag="den")
                nc.vector.tensor_tensor(out=den[:], in0=stats_all[0:T, fs],
                                        in1=e2[:], op=AOP.add)
                pe2 = fpool.tile([T, CH], f32, tag="pe2")
                nc.vector.tensor_tensor(out=pe2[:], in0=p2s[:], in1=e2[:],
                                        op=AOP.mult)
                num = fpool.tile([T, CH], f32, tag="num")
                nc.vector.tensor_tensor(out=num[:], in0=spe1[:], in1=pe2[:],
                                        op=AOP.add)
                rden = fpool.tile([T, CH], f32, tag="rden")
                nc.vector.reciprocal_approx_fast(out=rden[:], in_=den[:])
                sT = fpool.tile([T, CH], f32, tag="sT")
                nc.vector.tensor_tensor(out=sT[:], in0=num[:], in1=rden[:],
                                        op=AOP.mult)
                # sigmoid(s+b)*mask = (tanh((s+b)/2) + 1) * halfmask
                th = fpool.tile([T, CH], f32, tag="th")
                nc.scalar.activation(out=th[:], in_=sT[:], func=AF.Tanh,
                                     bias=bhalf[:], scale=0.5)
                nc.vector.scalar_tensor_tensor(
                    out=out_all[:, fs], in0=th[:], scalar=1.0, in1=hm_bc,
                    op0=AOP.add, op1=AOP.mult)

            # deferred per-group epilogue: exp/pe1/stats/evac/p2/finalize of
            # group g run interleaved into group g+1's PE stream so the PE
            # never stalls waiting for the ACT exp at a group boundary.
            def drain_prev(pv):
                gi_p = pv["gi"]
                nc.scalar.activation(
                    out=pv["e1pe1"][:, :, 0:T], in_=pv["p1q"][:], func=AF.Exp
                )
                nc.vector.tensor_tensor(
                    out=pv["e1pe1"][:, :, T:2 * T], in0=pv["p1q"][:],
                    in1=pv["e1pe1"][:, :, 0:T], op=AOP.mult,
                )
                for t in range(KT):
                    nc.tensor.matmul(
                        out=pv["acc"][0:17, P:2 * P], lhsT=pv["e1pe1"][:, t, :],
                        rhs=pv["oh"][:, t, :],
                        start=(t == 0), stop=(t == KT - 1),
                    )
                relu_aggT = fpool.tile([P, P], bf16, tag="relu_aggT")
                nc.scalar.activation(out=relu_aggT[:], in_=pv["acc"][:, 0:P],
                                     func=AF.Relu)
                nc.scalar.activation(out=stats_all[:, gi_p * P:(gi_p + 1) * P],
                                     in_=pv["acc"][0:17, P:2 * P], func=AF.Copy)
                p2T = ps_fin.tile([T, P], f32, tag="p2T")
                nc.tensor.matmul(
                    out=p2T[:], lhsT=wt[:], rhs=relu_aggT[:], start=True,
                    stop=True,
                )
                nc.scalar.activation(out=p2_all[:, gi_p * P:(gi_p + 1) * P],
                                     in_=p2T[:], func=AF.Copy)
                if gi_p < 16:
                    if (gi_p + 1) % CG == 0:
                        finalize_chunk(gi_p // CG * 512, 512)
                elif gi_p == 17:
                    finalize_chunk(16 * P, 2 * P)
                elif gi_p >= 18:
                    finalize_chunk(gi_p * P, P)

            prev = None
            pending_p1: list[tuple] = []  # (hTq_tile, p1q_tile, t)
            qrr = [0]  # global gather queue round-robin
            for gi in range(G):
                # ---- group loads ----
                edge_g = epool.tile([P, KT, D], bf16)
                nc.sync.dma_start(out=edge_g[:], in_=edge_d[:, gi, :, :])

                # one-hot S for the whole group in ONE DVE op:
                # S[e, t, n] = (iota[e, n] == dstloc[e, t]) via broadcast APs
                oh_g = opool.tile([P, KT, P], bf16)
                dl_ap = dstloc_all[:, gi * KT:(gi + 1) * KT]
                dl_b = dataclasses.replace(
                    dl_ap, ap=[dl_ap.ap[0], dl_ap.ap[1], [0, P]])
                io_ap = iota[:]
                io_b = dataclasses.replace(
                    io_ap, ap=[io_ap.ap[0], [0, KT], io_ap.ap[1]])
                nc.vector.tensor_tensor(out=oh_g[:], in0=io_b, in1=dl_b,
                                        op=AOP.is_equal)

                gath_g = gpool.tile([P, KT, D], bf16)
                msg_g = mpool.tile([P, KT, D], bf16)
                # SWDGE descriptor ring fits <1024 descs per DMA: chunk to 7
                # tiles (896 gather descriptors) per dma_gather; msg = gath +
                # edge runs per chunk so PE work starts as soon as chunk 0
                # lands.
                for ci, c0 in enumerate(range(0, KT, 7)):
                    cn = min(7, KT - c0)
                    nc.gpsimd.dma_gather(
                        gath_g[:, c0:c0 + cn, :],
                        srcemb_d[:],
                        srcidx_all[:, gi * (L // 16) + c0 * 8:
                                   gi * (L // 16) + (c0 + cn) * 8],
                        cn * P,
                        cn * P,
                        D,
                        queue_num=qrr[0] % 4,
                    )
                    qrr[0] += 1
                    nc.vector.tensor_tensor(
                        out=msg_g[:, c0:c0 + cn, :], in0=gath_g[:, c0:c0 + cn, :],
                        in1=edge_g[:, c0:c0 + cn, :], op=AOP.add)

                # one PSUM tile for both accumulators (saves a bank each):
                # cols 0:P = aggT [d, n] sum(msg); cols P:2P rows 0:17 =
                # statsT (rows 0:8 sum(e1)T, 8:16 sum(p1*e1)T, 16 deg)
                acc = ps_acc.tile([P, 2 * P], f32, tag="acc")
                p1q = ps_p1.tile([P, KT, T], f32)
                e1pe1 = wpool.tile([P, KT, 2 * T + 1], bf16, tag="e1pe1")
                nc.vector.memset(e1pe1[:, :, 2 * T:2 * T + 1], 1.0)
                hTq = wpool.tile([P, KT, P], bf16, tag="hTq")
                # p1 matmuls for quad q are emitted during quad q+1 so the
                # ACT relu of quad q hides under quad q+1's PE streams.
                for h0 in range(0, KT, 4):
                    hn = min(4, KT - h0)
                    msgT = ps_msg.tile([P, hn, P], f32, tag="msgT")
                    for t in range(h0, h0 + hn):
                        # msgT[d, e] = msg.T (transpose via ident)
                        nc.tensor.matmul(
                            out=msgT[:, t - h0, :], lhsT=msg_g[:, t, :],
                            rhs=ident[:], start=True, stop=True,
                        )
                        # aggT[d, n] += msg.T @ S (shared lhsT with transpose)
                        nc.tensor.matmul(
                            out=acc[:, 0:P], lhsT=msg_g[:, t, :],
                            rhs=oh_g[:, t, :],
                            start=(t == 0), stop=(t == KT - 1),
                        )
                        if pending_p1:
                            hq, pq, tp = pending_p1.pop(0)
                            nc.tensor.matmul(
                                out=pq[:, tp, :], lhsT=hq[:, tp, :], rhs=wt[:],
                                start=True, stop=True,
                            )
                    if h0 == 0 and prev is not None:
                        drain_prev(prev)
                        prev = None
                    # hT = relu(msgT) per quad (ACT, PSUM->SBUF)
                    nc.scalar.activation(out=hTq[:, h0:h0 + hn, :],
                                         in_=msgT[:], func=AF.Relu)
                    pending_p1.extend(
                        (hTq, p1q, t) for t in range(h0, h0 + hn))
                prev = {"gi": gi, "oh": oh_g, "e1pe1": e1pe1, "p1q": p1q,
                        "acc": acc}

            for hq, pq, tp in pending_p1:
                nc.tensor.matmul(
                    out=pq[:, tp, :], lhsT=hq[:, tp, :], rhs=wt[:],
                    start=True, stop=True,
                )
            pending_p1.clear()
            drain_prev(prev)

            nc.sync.dma_start(out=out_d[:], in_=out_all[:])

    nc.compile()
    return nc


def _host_prep(src_embedding, edge_embedding, W, b, src, dst):
    """Sort/pad/shard edges on the host; returns (KT, in_maps)."""
    src = np.asarray(src).astype(np.int64)
    dst = np.asarray(dst).astype(np.int64)
    edge_embedding = np.asarray(edge_embedding, dtype=np.float32)
    src_embedding = np.asarray(src_embedding, dtype=np.float32)
    W = np.asarray(W, dtype=np.float32)
    b = np.asarray(b, dtype=np.float32)

    # ---- balance edge counts across the 160 (core, group) bins by
    # permuting the node->group assignment (LPT greedy on node in-degree):
    # KT is set by the heaviest group, so balancing trims ~6% of all
    # per-tile work including the pacing gather stream ----
    import heapq

    deg_n = np.bincount(dst, minlength=N_NODES)
    node_order = np.argsort(-deg_n, kind="stable")
    heap = [(0, g) for g in range(N_GROUPS)]
    heapq.heapify(heap)
    nslot = np.zeros(N_GROUPS, dtype=np.int64)
    g_of_node = np.empty(N_NODES, dtype=np.int64)
    slot_of_node = np.empty(N_NODES, dtype=np.int64)
    spill = []
    for n in node_order:
        load, g = heapq.heappop(heap)
        g_of_node[n] = g
        slot_of_node[n] = nslot[g]
        nslot[g] += 1
        item = (load + int(deg_n[n]), g)
        if nslot[g] < P:
            heapq.heappush(heap, item)
        else:
            spill.append(item)
        if not heap:
            heap = spill
            heapq.heapify(heap)
            spill = []
    node_of = np.full((N_GROUPS, P), -1, dtype=np.int64)
    node_of[g_of_node, slot_of_node] = np.arange(N_NODES)

    grp = g_of_node[dst]
    order = np.argsort(grp, kind="stable")
    s_src = src[order]
    s_dstslot = slot_of_node[dst[order]]
    s_edge = edge_embedding[order]

    counts = np.bincount(grp, minlength=N_GROUPS)
    KT = max(1, int(-(-counts.max() // P)))  # ceil / 128
    L = KT * P
    offs = np.concatenate([[0], np.cumsum(counts)])

    edge_c = np.zeros((CORES, P, G, KT, D), dtype=BF16)
    dstloc_c = np.full((CORES, P, G * KT), PAD_DST, dtype=BF16)
    srcidx_c = np.zeros((CORES, P, G * L // 16), dtype=np.int16)

    for g in range(N_GROUPS):
        c, gi = divmod(g, G)
        o0, o1 = int(offs[g]), int(offs[g + 1])
        cnt = o1 - o0
        if cnt:
            blk = np.zeros((L, D), dtype=BF16)
            blk[:cnt] = s_edge[o0:o1].astype(BF16)
            # edge t*128+p -> [p, t, :]
            edge_c[c, :, gi, :, :] = blk.reshape(KT, P, D).transpose(1, 0, 2)
        dl = np.full(L, PAD_DST, dtype=np.float32)
        dl[:cnt] = s_dstslot[o0:o1].astype(np.float32)
        # tile layout: edge t*128+p -> [p, t]
        dstloc_c[c, :, gi * KT:(gi + 1) * KT] = dl.reshape(KT, P).T.astype(BF16)
        ids = np.zeros(L, dtype=np.int16)
        ids[:cnt] = s_src[o0:o1].astype(np.int16)
        # dma_gather index layout: logical i -> [i % 16, i // 16], x8 replicas
        wrapped = ids.reshape(L // 16, 16).T  # [16, L//16]
        srcidx_c[c, :, gi * (L // 16):(gi + 1) * (L // 16)] = np.tile(
            wrapped, (8, 1)
        )

    consts = {
        "srcemb": src_embedding.astype(BF16),
        "ident": np.eye(P, dtype=np.float32).astype(BF16),
        "iota": np.tile(np.arange(P, dtype=np.float32), (P, 1)).astype(BF16),
        "wt": W.T.copy().astype(BF16),
        "ones18": np.ones((1, T), dtype=np.float32).astype(BF16),
        "bhalf": (b / 2.0).reshape(T, 1).astype(np.float32),
    }
    in_maps = [
        {
            "edge": edge_c[c],
            "dstloc": dstloc_c[c],
            "srcidx": srcidx_c[c],
            **consts,
        }
        for c in range(CORES)
    ]
    return KT, in_maps, node_of


def kernel(src_embedding, edge_embedding, W, b, src, dst):
    global LAST_RESULT
    KT, in_maps, node_of = _host_prep(src_embedding, edge_embedding, W, b,
                                      src, dst)
    run = _get_runner(KT)
    outs = run(in_maps)
    LAST_RESULT = None
    flat = np.empty((N_PAD, T), dtype=np.float32)
    for c in range(CORES):
        blk = np.asarray(outs[c]["out"], dtype=np.float32)
        flat[c * NODES_PER_CORE:(c + 1) * NODES_PER_CORE] = blk.T
    nodes = node_of.ravel()
    valid = nodes >= 0
    out = np.empty((N_NODES, T), dtype=np.float32)
    out[nodes[valid]] = flat[valid]
    return out


class _Runner:
    """Cached PJRT executor for a compiled Bass module (mirrors
    bass2jax.run_bass_via_pjrt but keeps the jitted callable + device inputs
    so repeated calls don't re-lower, and so timing loops are possible)."""

    def __init__(self, nc):
        import jax
        from jax.sharding import Mesh, PartitionSpec
        from jax.experimental.shard_map import shard_map
        import concourse.mybir as mybir
        from concourse import bass2jax

        bass2jax.install_neuronx_cc_hook()
        self.nc = nc
        in_names, out_names, out_avals, zero_outs = [], [], [], []
        for alloc in nc.m.functions[0].allocations:
            if not isinstance(alloc, mybir.MemoryLocationSet):
                continue
            name = alloc.memorylocations[0].name
            if alloc.kind == "ExternalInput":
                in_names.append(name)
            elif alloc.kind == "ExternalOutput":
                out_names.append(name)
                shape = tuple(alloc.tensor_shape)
                dtype = mybir.dt.np(alloc.dtype)
                out_avals.append(jax.core.ShapedArray(shape, dtype))
                zero_outs.append(np.zeros(shape, dtype))
        assert nc.partition_id_tensor is None, "partition id unused"
        self.in_names = list(in_names)
        self.out_names = out_names
        self.out_avals = out_avals
        self.zero_outs = zero_outs
        n_params = len(in_names)
        n_outs = len(out_avals)
        all_in_names = in_names + out_names
        donate = tuple(range(n_params, n_params + n_outs))

        def _body(*args):
            outs = bass2jax._bass_exec_p.bind(
                *args,
                out_avals=tuple(out_avals),
                in_names=tuple(all_in_names),
                out_names=tuple(out_names),
                lowering_input_output_aliases=(),
                sim_require_finite=True,
                sim_require_nnan=True,
                nc=nc,
            )
            return tuple(outs)

        devices = jax.devices()[:CORES]
        self.mesh = Mesh(np.asarray(devices), ("core",))
        in_specs = (PartitionSpec("core"),) * (n_params + n_outs)
        out_specs = (PartitionSpec("core"),) * n_outs
        self.fn = jax.jit(
            shard_map(_body, mesh=self.mesh, in_specs=in_specs,
                      out_specs=out_specs, check_rep=False),
            donate_argnums=donate, keep_unused=True,
        )
        self._dev_inputs = None

    def set_inputs(self, in_maps):
        import jax
        from jax.sharding import NamedSharding, PartitionSpec

        concat_in = [
            np.concatenate([np.asarray(in_maps[c][name]) for c in range(CORES)],
                           axis=0)
            for name in self.in_names
        ]
        sh = NamedSharding(self.mesh, PartitionSpec("core"))
        self._dev_inputs = [jax.device_put(a, sh) for a in concat_in]

    def execute(self):
        """One NEFF execution (inputs already on device). Returns jax arrays."""
        import jax
        from jax.sharding import NamedSharding, PartitionSpec

        sh = NamedSharding(self.mesh, PartitionSpec("core"))
        zeros = [
            jax.device_put(np.zeros((CORES * z.shape[0], *z.shape[1:]), z.dtype), sh)
            for z in self.zero_outs
        ]
        out = self.fn(*self._dev_inputs, *zeros)
        jax.block_until_ready(out)
        return out

    def __call__(self, in_maps):
        self.set_inputs(in_maps)
        out_arrs = self.execute()
        return [
            {
                name: np.asarray(out_arrs[i]).reshape(
                    CORES, *self.out_avals[i].shape)[c]
                for i, name in enumerate(self.out_names)
            }
            for c in range(CORES)
        ]


def _get_runner(KT: int) -> _Runner:
    run = _BUILD_CACHE.get(KT)
    if run is None:
        run = _Runner(_build(KT))
        _BUILD_CACHE[KT] = run
    return run
